# revision 6
# baseline (speedup 1.0000x reference)
"""Causal self-attention with RoPE on 8 Trainium2 NeuronCores.

Sharding: batch (4) x head-group (2 groups of 8 heads) -> 8 cores.
Each core computes, for its (batch b, head group g):
  qkv projection (fp16 matmuls), RoPE (DVE/gpsimd + DMA partition shift),
  causal attention (scores per 128-key block for both heads of a pair in
  one 2-bank PSUM tile, exp on ACT with folded 1/8 scale, probs/V fp16,
  exact-sum normalization via ones-column), output projection partial.
Host sums the two head-group partials per batch.

v3 schedule: x resident in SBUF; qk-proj emitted sc-chunk-major and
interleaved with attention chunks per query block t so the PE fills
ACT-bound softmax gaps with projection matmuls; attention inner loop
software-pipelined (scores one jp ahead of exp/pav).
"""
import sys

sys.path.insert(0, "/opt/trn_rl_repo")

import numpy as np

import concourse.bass as bass  # noqa: F401
import concourse.mybir as mybir
import concourse.tile as tile
from concourse import bacc
from concourse.bass_utils import run_bass_kernel_spmd

dt = mybir.dt
F32, F32R, F16 = dt.float32, dt.float32r, dt.float16
ALU = mybir.AluOpType
EXP = mybir.ActivationFunctionType.Exp

ROPE_BASE = 10000.0


def build_core_program(S=2048, D=1024, HL=8, hd=64):
    """Bass program for one core (see module docstring)."""
    assert hd == 64
    NP = HL // 2          # head pairs
    DT = D // 128         # k-tiles of the D contraction
    SC = S // 512         # 512-query chunks
    ST = S // 128         # 128-row tiles of S
    NJT = 2 * NP          # q/k j-tiles (2 heads * 64 = 128 cols each)
    scale = hd ** -0.5

    nc = bacc.Bacc("TRN2", target_bir_lowering=False, debug=False)
    xT_d = nc.dram_tensor("xT", [D, S], F16, kind="ExternalInput").ap()
    Wqk_d = nc.dram_tensor("Wqk", [D, NJT * 128], F16, kind="ExternalInput").ap()
    Wv_d = nc.dram_tensor("Wv", [D, HL * hd], F16, kind="ExternalInput").ap()
    Wout_d = nc.dram_tensor("Wout", [HL * hd, D], F16, kind="ExternalInput").ap()
    cos_d = nc.dram_tensor("cosT", [128, S], F16, kind="ExternalInput").ap()
    s2_d = nc.dram_tensor("S2T", [128, S], F16, kind="ExternalInput").ap()
    tri_d = nc.dram_tensor("tri", [128, 128], F16, kind="ExternalInput").ap()
    y_d = nc.dram_tensor("y", [S, D], F32, kind="ExternalOutput").ap()

    with tile.TileContext(nc) as tc:
        with tc.tile_pool(name="persist", bufs=1) as pp, \
             tc.tile_pool(name="rope", bufs=4) as rp, \
             tc.tile_pool(name="expp", bufs=4) as expp, \
             tc.tile_pool(name="normp", bufs=2) as normp, \
             tc.tile_pool(name="proj", bufs=2, space="PSUM") as pjp, \
             tc.tile_pool(name="pavp", bufs=1, space="PSUM") as pavp, \
             tc.tile_pool(name="sps", bufs=2, space="PSUM") as sps:

            # ---- persistent SBUF ----
            x_sb = pp.tile([128, DT, S], F16, tag="x_sb")
            qkT = [pp.tile([128, S], F16, tag=f"qkT{j}", name=f"qkT{j}")
                   for j in range(NJT)]
            v_sb = pp.tile([128, ST, HL, 66], F16, tag="v_sb")
            outT = [pp.tile([128, S], F16, tag=f"outT{p}", name=f"outT{p}")
                    for p in range(NP)]
            cosT = pp.tile([128, S], F16, tag="cosT")
            s2T = pp.tile([128, S], F16, tag="s2T")
            tri2 = pp.tile([128, 2, 128], F16, tag="tri2")
            wv = pp.tile([128, DT, 512], F16, tag="wv")
            wqk = pp.tile([128, DT, NJT, 128], F16, tag="wqk")
            wout = pp.tile([128, NP, D // 512, 512], F16, tag="wout")

            # ---- input DMAs (scalar queue: ACT idle early) ----
            nc.scalar.dma_start(out=cosT[:], in_=cos_d[:])
            nc.scalar.dma_start(out=s2T[:], in_=s2_d[:])
            for jj in range(2):
                nc.scalar.dma_start(out=tri2[:, jj, :], in_=tri_d[:])
            for ddt in range(DT):
                nc.scalar.dma_start(
                    out=x_sb[:, ddt, :],
                    in_=xT_d[ddt * 128:(ddt + 1) * 128, :])
                nc.scalar.dma_start(
                    out=wv[:, ddt, :], in_=Wv_d[ddt * 128:(ddt + 1) * 128, :])
                nc.scalar.dma_start(
                    out=wqk[:, ddt, :, :],
                    in_=Wqk_d[ddt * 128:(ddt + 1) * 128, :])
            for p in range(NP):
                for dc in range(D // 512):
                    nc.scalar.dma_start(
                        out=wout[:, p, dc, :],
                        in_=Wout_d[p * 128:(p + 1) * 128,
                                   dc * 512:(dc + 1) * 512])
            nc.vector.memset(v_sb[:, :, :, 64:65], 1.0)

            # ---- v projection for one 512-row chunk of S ----
            def v_proj(sc):
                for stl in range(4):
                    st = sc * 4 + stl
                    vps = pjp.tile([128, 512], F32, tag="proj", name="vps")
                    for ddt in range(DT):
                        nc.tensor.matmul(
                            vps[:],
                            x_sb[:, ddt, st * 128:(st + 1) * 128],
                            wv[:, ddt, :], start=(ddt == 0),
                            stop=(ddt == DT - 1))
                    nc.vector.tensor_copy(
                        v_sb[:, st, :, 0:64],
                        vps[:].rearrange("p (h c) -> p h c", h=HL))

            # ---- qk projection + RoPE for j-tile jt, 512-col chunk sc ----
            def qk_group(jt, sc):
                qkps = pjp.tile([128, 512], F32, tag="proj", name="qkps")
                for ddt in range(DT):
                    nc.tensor.matmul(
                        qkps[:], wqk[:, ddt, jt, :],
                        x_sb[:, ddt, sc * 512:(sc + 1) * 512],
                        start=(ddt == 0), stop=(ddt == DT - 1))
                ss = slice(sc * 512, (sc + 1) * 512)
                t16 = rp.tile([128, 512], F16, tag="t16", name="t16")
                nc.vector.tensor_copy(t16[:], qkps[:])
                nc.vector.tensor_tensor(
                    qkT[jt][:, ss], t16[:], cosT[:, ss], ALU.mult)
                rot = rp.tile([128, 512], F16, tag="rot", name="rot")
                for b0 in range(4):
                    src = (b0 ^ 1) * 32
                    nc.sync.dma_start(
                        out=rot[b0 * 32:(b0 + 1) * 32, :],
                        in_=t16[src:src + 32, :])
                rotm = rp.tile([128, 512], F16, tag="rotm", name="rotm")
                nc.gpsimd.tensor_tensor(
                    rotm[:], rot[:], s2T[:, ss], ALU.mult)
                nc.vector.tensor_tensor(
                    qkT[jt][:, ss], qkT[jt][:, ss], rotm[:], ALU.add)

            # ---- attention for head pair p, 512-query chunk t ----
            def attn_chunk(p, t):
                qT, kT = qkT[2 * p], qkT[2 * p + 1]
                njp = 4 * t + 4          # 128-key blocks
                pav = [pavp.tile([128, 512], F32, tag=f"pav{hh}",
                                 name="pav") for hh in range(2)]
                spts, ets = {}, {}

                def emit_scores(jp):
                    spt = sps.tile([128, 2, 512], F32, tag="sps", name="spt")
                    for hh in range(2):
                        hb = 64 * hh
                        nc.tensor.matmul(
                            spt[:, hh, :],
                            kT[hb:hb + 64, jp * 128:(jp + 1) * 128],
                            qT[hb:hb + 64, t * 512:(t + 1) * 512],
                            start=True, stop=True)
                    spts[jp] = spt

                def emit_exp(jp):
                    spt = spts.pop(jp)
                    d = jp - 4 * t
                    et = expp.tile([128, 2, 512], F16, tag="expp", name="et")
                    if d < 0:
                        nc.scalar.activation(et[:], spt[:], EXP, scale=scale)
                    else:
                        if d > 0:
                            nc.vector.memset(et[:, :, 0:d * 128], 0.0)
                        nc.scalar.activation(
                            et[:, :, d * 128:512], spt[:, :, d * 128:512],
                            EXP, scale=scale)
                        nc.vector.tensor_tensor(
                            et[:, :, d * 128:(d + 1) * 128],
                            et[:, :, d * 128:(d + 1) * 128],
                            tri2[:], ALU.mult)
                    ets[jp] = et

                def emit_pav(jp):
                    et = ets.pop(jp)
                    for hh in range(2):
                        h = 2 * p + hh
                        nc.tensor.matmul(
                            pav[hh][0:65, :],
                            v_sb[:, jp, h, 0:65],
                            et[:, hh, :],
                            start=(jp == 0), stop=(jp == njp - 1))

                emit_scores(0)
                for jp in range(njp):
                    if jp + 1 < njp:
                        emit_scores(jp + 1)
                    emit_exp(jp)
                    emit_pav(jp)
                for hh in range(2):
                    srow = normp.tile([1, 512], F32, tag="srow", name="srow")
                    nc.scalar.copy(srow[:], pav[hh][64:65, :])
                    rstage = normp.tile([1, 512], F32, tag="rst", name="rst")
                    scr = normp.tile([1, 512], F32, tag="scr", name="scr")
                    nc.vector.reciprocal_approx_accurate(
                        out=rstage[:], in_=srow[:], scratch=scr[:])
                    brec = normp.tile([64, 512], F32, tag="brec", name="brec")
                    nc.gpsimd.partition_broadcast(brec[:], rstage[:])
                    nc.vector.tensor_tensor(
                        outT[p][64 * hh:64 * hh + 64,
                                t * 512:(t + 1) * 512],
                        pav[hh][0:64, :], brec[:], ALU.mult)

            # ---- output projection for 512-query chunk t ----
            def out_proj(t):
                NDC = D // 512
                for stl in range(4):
                    st = t * 4 + stl
                    for dc in range(NDC):
                        yp2 = pjp.tile([128, 512], F32, tag="proj", name="yps")
                        for p in range(NP):
                            nc.tensor.matmul(
                                yp2[:], outT[p][:, st * 128:(st + 1) * 128],
                                wout[:, p, dc, :],
                                start=(p == 0), stop=(p == NP - 1))
                        yst = normp.tile([128, 512], F32, tag="yst",
                                         name="yst")
                        nc.scalar.copy(yst[:], yp2[:])
                        nc.gpsimd.dma_start(
                            out=y_d[st * 128:(st + 1) * 128,
                                    dc * 512:(dc + 1) * 512],
                            in_=yst[:])

            # ---- v3 interleaved schedule ----
            for jt in range(NJT):
                qk_group(jt, 0)
            v_proj(0)
            for p in range(NP):
                attn_chunk(p, 0)
            for jt in range(NJT):
                qk_group(jt, 1)
            v_proj(1)
            for p in range(NP):
                attn_chunk(p, 1)
            for jt in range(NJT):
                qk_group(jt, 2)
            v_proj(2)
            out_proj(0)
            for p in range(NP):
                attn_chunk(p, 2)
            for jt in range(NJT):
                qk_group(jt, 3)
            v_proj(3)
            out_proj(1)
            for p in range(NP):
                attn_chunk(p, 3)
            out_proj(2)
            out_proj(3)
    nc.compile()
    return nc


def make_tables(S=2048, hd=64):
    inv_freq = 1.0 / (ROPE_BASE ** (np.arange(0, hd, 2, dtype=np.float64) / hd))
    t = np.arange(S, dtype=np.float64)
    freqs = np.outer(t, inv_freq)                    # [S, 32]
    emb = np.concatenate([freqs, freqs], axis=-1)    # [S, 64]
    cos1 = np.cos(emb).T.astype(np.float32)          # [64, S]
    sin1 = np.sin(emb).T.astype(np.float32)
    s2_1 = sin1.copy()
    s2_1[0:32] = -s2_1[0:32]
    cosT = np.concatenate([cos1, cos1], axis=0).astype(np.float16)  # [128, S]
    s2T = np.concatenate([s2_1, s2_1], axis=0).astype(np.float16)
    tri = np.tril(np.ones((128, 128), np.float32)).T.astype(np.float16)
    # tri[j, i] = 1 iff j <= i  (lower-tri transposed = upper-tri in [j, i])
    return cosT, s2T, tri


def make_core_inputs(x, Wqkv, Wout, b, g, HL=8, hd=64):
    """Host-side shard prep for core (batch b, head group g)."""
    B, S, D = x.shape
    H = D // hd
    heads = list(range(g * HL, (g + 1) * HL))
    Wq = Wqkv[:, 0:D].reshape(D, H, hd)
    Wk = Wqkv[:, D:2 * D].reshape(D, H, hd)
    Wv = Wqkv[:, 2 * D:3 * D].reshape(D, H, hd)
    # Wqk j-tile order: q(h0,h1), k(h0,h1), q(h2,h3), k(h2,h3), ...
    blocks = []
    for p in range(HL // 2):
        h0, h1 = heads[2 * p], heads[2 * p + 1]
        blocks.append(np.concatenate([Wq[:, h0], Wq[:, h1]], axis=1))
        blocks.append(np.concatenate([Wk[:, h0], Wk[:, h1]], axis=1))
    Wqk_host = np.ascontiguousarray(np.concatenate(blocks, axis=1), np.float16)
    Wv_host = np.ascontiguousarray(
        Wv[:, heads].reshape(D, HL * hd), np.float16)
    Wout_host = np.ascontiguousarray(
        Wout[g * HL * hd:(g + 1) * HL * hd, :], np.float16)
    xT = np.ascontiguousarray(x[b].T, np.float16)
    cosT, s2T, tri = make_tables(S, hd)
    return {"xT": xT, "Wqk": Wqk_host, "Wv": Wv_host, "Wout": Wout_host,
            "cosT": cosT, "S2T": s2T, "tri": tri}


_NC_CACHE = {}
TRACE = False          # test-only: capture NTFF profile + exec time
LAST_EXEC_NS = None
LAST_RESULT = None


def _enable_ntff_hook():
    import types
    import trn_agent_boot.trn_boot as tb
    import concourse.bass_utils as bu
    m = types.ModuleType("antenv.axon_hooks")
    _hook = [None]
    m.set_axon_ntff_profile_hook = lambda h: _hook.__setitem__(0, h)
    m.get_axon_ntff_profile_hook = lambda: _hook[0]
    sys.modules["antenv.axon_hooks"] = m
    m.set_axon_ntff_profile_hook(
        tb._ntff_profile_via_ctypes("/opt/axon/libaxon_pjrt.so"))
    bu.upload_artifacts = lambda tmpdir: ""


def kernel(x, Wqkv, Wout):
    global LAST_EXEC_NS, LAST_RESULT
    B, S, D = x.shape
    key = (B, S, D)
    if key not in _NC_CACHE:
        _NC_CACHE[key] = build_core_program(S=S, D=D)
    nc = _NC_CACHE[key]
    in_maps = []
    for core in range(8):
        b, g = core // 2, core % 2
        in_maps.append(make_core_inputs(np.asarray(x), np.asarray(Wqkv),
                                        np.asarray(Wout), b, g))
    kw = {}
    if TRACE:
        _enable_ntff_hook()
        kw = dict(trace=True, trace_cores=[0])
    res = run_bass_kernel_spmd(nc, in_maps, core_ids=list(range(8)), **kw)
    LAST_EXEC_NS = res.exec_time_ns
    LAST_RESULT = res
    y = np.empty((B, S, D), np.float32)
    for b in range(B):
        y[b] = res.results[2 * b]["y"] + res.results[2 * b + 1]["y"]
    return y
